# revision 5
# baseline (speedup 1.0000x reference)
"""Trainium2 Bass kernel for nn_Policy_31104153158020.

Policy network: token parse -> scatter into [22,11,11] grid -> CNN trunk
(conv 5x5 s3 -> conv 3x3 -> fc) + self encoder on center cell -> 3 heads.

Strategy (pure data parallel over 8 cores, B=16384 -> 2048 rows/core):
- Parse tokens on DVE (bit ops) in groups of 4 row-tiles.
- Scatter per 128-row tile with GPSIMD local_scatter (two overlapping
  halves; HW processes indices in token order => last-write-wins,
  matching XLA scatter semantics exactly).
- conv1 as 45 strided PE transposes (im2col chunks [a,dx] x 5 dy) feeding
  45 fp16 matmuls; conv2/fc/self/heads as plain matmuls; normalization
  (1/max_vec) folded into conv1/self weights on the host.
"""
import os
import sys

sys.path.insert(0, "/opt/trn_rl_repo")

import numpy as np

import concourse.bass as bass
import concourse.bacc as bacc
import concourse.mybir as mybir
import concourse.tile as tile
from concourse import bass_utils
from concourse.ap import AP
from concourse.alu_op_type import AluOpType as op

N_CORES = 8
B = 16384
M = 200
B_C = B // N_CORES          # 2048 rows per core
TILES = B_C // 128          # 16 tiles of 128 rows
GROUP = 4                   # tiles parsed together
N_GROUPS = TILES // GROUP

NUM_LAYERS, OW, OH = 22, 11, 11
NCELL = NUM_LAYERS * OW * OH          # 2662
# scatter split: A covers cells [0,1332) (+dump 1332), B covers [1332,2662)
NE_A = 1334
NE_B = 1334
BOXW = 1332 + NE_B                    # 2666 box columns; box[:, c] = cell c

F16 = mybir.dt.float16
F32 = mybir.dt.float32
I16 = mybir.dt.int16
I32 = mybir.dt.int32

_MAX_VEC = np.maximum(np.array([9.0, 1.0, 1.0, 10.0, 3.0, 254.0, 1.0, 1.0, 235.0, 8.0,
                                9.0, 250.0, 29.0, 1.0, 1.0, 8.0, 1.0, 1.0, 6.0, 3.0,
                                1.0, 2.0], dtype=np.float32), 1.0)
_DIV = _MAX_VEC + 1e-8


def _build():
    nc = bacc.Bacc("TRN2", target_bir_lowering=False, debug=False,
                   num_devices=N_CORES)

    obs_d = nc.dram_tensor("obs", [B_C, 3 * M], I32, kind="ExternalInput").ap()
    w1_d = nc.dram_tensor("w1c", [110, 640], F16, kind="ExternalInput").ap()
    w2_d = nc.dram_tensor("w2c", [128, 1152], F16, kind="ExternalInput").ap()
    fc_d = nc.dram_tensor("fcwc", [128, 256], F16, kind="ExternalInput").ap()
    sw_d = nc.dram_tensor("selfwc", [22, 256], F16, kind="ExternalInput").ap()
    hw_d = nc.dram_tensor("headwc", [128, 80], F16, kind="ExternalInput").ap()
    id_d = nc.dram_tensor("identc", [128, 128], F16, kind="ExternalInput").ap()
    bc1_d = nc.dram_tensor("bc1", [128, 1], F32, kind="ExternalInput").ap()
    bc2_d = nc.dram_tensor("bc2", [128, 1], F32, kind="ExternalInput").ap()
    bfc_d = nc.dram_tensor("bfc", [128, 2], F32, kind="ExternalInput").ap()
    bself_d = nc.dram_tensor("bself", [128, 2], F32, kind="ExternalInput").ap()
    bhead_d = nc.dram_tensor("bhead", [20, 1], F32, kind="ExternalInput").ap()
    out_d = nc.dram_tensor("out", [B_C, 20], F32, kind="ExternalOutput").ap()

    with tile.TileContext(nc) as tc:
        with (
            tc.tile_pool(name="const", bufs=1) as pc,
            tc.tile_pool(name="grp", bufs=2) as pg,
            tc.tile_pool(name="tok", bufs=2) as pt,
            tc.tile_pool(name="boxp", bufs=2) as pb,
            tc.tile_pool(name="imc", bufs=2) as pi,
            tc.tile_pool(name="act", bufs=2) as pa,
            tc.tile_pool(name="psT", bufs=2, space="PSUM") as psT,
            tc.tile_pool(name="psC", bufs=1, space="PSUM") as psC,
            tc.tile_pool(name="psS", bufs=3, space="PSUM") as psS,
        ):
            w1_t = pc.tile([110, 640], F16)
            w2_t = pc.tile([128, 1152], F16)
            fc_t = pc.tile([128, 256], F16)
            sw_t = pc.tile([22, 256], F16)
            hw_t = pc.tile([128, 80], F16)
            id_t = pc.tile([128, 128], F16)
            bc1_t = pc.tile([128, 1], F32)
            bc2_t = pc.tile([128, 1], F32)
            bfc_t = pc.tile([128, 2], F32)
            bself_t = pc.tile([128, 2], F32)
            bhead_t = pc.tile([20, 1], F32)
            nc.sync.dma_start(w1_t[:], w1_d)
            nc.sync.dma_start(w2_t[:], w2_d)
            nc.sync.dma_start(fc_t[:], fc_d)
            nc.sync.dma_start(sw_t[:], sw_d)
            nc.sync.dma_start(hw_t[:], hw_d)
            nc.sync.dma_start(id_t[:], id_d)
            nc.sync.dma_start(bc1_t[:], bc1_d)
            nc.sync.dma_start(bc2_t[:], bc2_d)
            nc.sync.dma_start(bfc_t[:], bfc_d)
            nc.sync.dma_start(bself_t[:], bself_d)
            nc.sync.dma_start(bhead_t[:], bhead_d)

            for g in range(N_GROUPS):
                FD = GROUP * M
                obs_g = pg.tile([128, GROUP * 3 * M], I32, tag="obs")
                for j in range(GROUP):
                    t0 = g * GROUP + j
                    nc.sync.dma_start(
                        obs_g[:, j * 3 * M:(j + 1) * 3 * M],
                        obs_d[t0 * 128:(t0 + 1) * 128, :],
                    )

                # token fields, strided int32 -> int16 casts
                c16 = pt.tile([128, FD], I16, tag="c16")
                a16 = pt.tile([128, FD], I16, tag="a16")
                vals = pt.tile([128, FD], F16, tag="vals")
                obs_v = obs_g[:]
                nc.vector.tensor_copy(c16[:], obs_v[:, 0::3])
                nc.vector.tensor_copy(a16[:], obs_v[:, 1::3])
                nc.vector.tensor_copy(vals[:], obs_v[:, 2::3])

                y16 = pt.tile([128, FD], I16, tag="y16")
                x16 = pt.tile([128, FD], I16, tag="x16")
                t1 = pt.tile([128, FD], I16, tag="t1")
                cell = pt.tile([128, FD], I16, tag="cell")
                m0 = pt.tile([128, FD], I16, tag="m0")
                m1 = pt.tile([128, FD], I16, tag="m1")
                idxA = pt.tile([128, FD], I16, tag="idxA")
                idxB = pt.tile([128, FD], I16, tag="idxB")

                nc.vector.tensor_scalar(y16[:], c16[:], 15, None, op.bitwise_and)
                nc.vector.tensor_scalar(x16[:], c16[:], 4, None, op.logical_shift_right)
                # cell = 242*x + 11*a + y  (im2col-friendly permutation:
                # chunk (p,dy) = single stride-11 run over (dx, a))
                nc.vector.tensor_scalar(t1[:], x16[:], 242, None, op.mult)
                nc.vector.tensor_scalar(cell[:], a16[:], 11, None, op.mult)
                nc.vector.tensor_tensor(cell[:], cell[:], t1[:], op.add)
                nc.vector.tensor_tensor(cell[:], cell[:], y16[:], op.add)
                # validity mask
                nc.vector.tensor_scalar(m0[:], x16[:], 11, None, op.is_lt)
                nc.vector.tensor_scalar(m1[:], y16[:], 11, None, op.is_lt)
                nc.vector.tensor_tensor(m0[:], m0[:], m1[:], op.mult)
                nc.vector.tensor_scalar(m1[:], a16[:], 22, None, op.is_lt)
                nc.vector.tensor_tensor(m0[:], m0[:], m1[:], op.mult)
                # invalid -> cell + 16384 (wraps to negative for large cells)
                nc.vector.tensor_scalar(m0[:], m0[:], -16384, 16384, op.mult, op.add)
                nc.vector.tensor_tensor(cell[:], cell[:], m0[:], op.add)
                # split indices for the two scatter halves
                nc.vector.tensor_scalar(idxA[:], cell[:], 1332, None, op.min)
                nc.vector.tensor_scalar(idxB[:], cell[:], 2664, 1332, op.min, op.subtract)

                for j in range(GROUP):
                    t0 = g * GROUP + j
                    sl = slice(j * M, (j + 1) * M)
                    box = pb.tile([128, BOXW], F16, tag="box")
                    nc.gpsimd.local_scatter(
                        box[:, 0:NE_A], vals[:, sl], idxA[:, sl],
                        channels=128, num_elems=NE_A, num_idxs=M)
                    nc.gpsimd.local_scatter(
                        box[:, 1332:1332 + NE_B], vals[:, sl], idxB[:, sl],
                        channels=128, num_elems=NE_B, num_idxs=M)

                    box_ap = box[:]
                    part_dim = list(box_ap.ap)[0]

                    # --- self encoder on center cell (5,5): cells 121*a+60
                    ctr_ps = psS.tile([22, 128], F16, tag="small", name="ctr_ps")
                    ctr_in = AP(box_ap.tensor, box_ap.offset + 1215,
                                [part_dim, [11, 22]])
                    nc.tensor.transpose(ctr_ps[:], ctr_in, id_t[:])
                    ctr_s = pa.tile([22, 128], F16, tag="ctr_s")
                    nc.vector.tensor_copy(ctr_s[:], ctr_ps[:])

                    hid = pa.tile([128, 512], F16, tag="hid")
                    sf_ps = psS.tile([128, 256], F32, tag="small", name="sf_ps")
                    for h in range(2):
                        nc.tensor.matmul(
                            sf_ps[:, h * 128:(h + 1) * 128],
                            sw_t[:, h * 128:(h + 1) * 128], ctr_s[:],
                            start=True, stop=True)
                        nc.scalar.activation(
                            hid[:, h * 128:(h + 1) * 128],
                            sf_ps[:, h * 128:(h + 1) * 128],
                            mybir.ActivationFunctionType.Relu,
                            bias=bself_t[:, h:h + 1])

                    # --- conv1: 45 strided transposes + 45 matmuls
                    imc = [pi.tile([110, 512], F16, tag=f"imc{b}", name=f"imc{b}") for b in range(12)]
                    psb = None
                    c1_ps = [psC.tile([128, 512], F32, tag=f"c1_{b}", name=f"c1_{b}") for b in range(3)]
                    for k in range(45):
                        p, dy = divmod(k, 5)
                        ox, oy = divmod(p, 3)
                        if k % 4 == 0:
                            psb = psT.tile([110, 512], F16, tag="T")
                        chunk = AP(box_ap.tensor,
                                   box_ap.offset + 726 * ox + 3 * oy + dy,
                                   [part_dim, [11, 110]])
                        nc.tensor.transpose(
                            psb[:, (k % 4) * 128:(k % 4 + 1) * 128], chunk, id_t[:])
                        if k % 4 == 3 or k == 44:
                            b = k // 4
                            w = (k % 4 + 1) * 128
                            if b % 2 == 0:
                                nc.vector.tensor_copy(imc[b][:, :w], psb[:, :w])
                            else:
                                nc.scalar.copy(imc[b][:, :w], psb[:, :w])
                    for p in range(9):
                        for dy in range(5):
                            k = p * 5 + dy
                            rhs = imc[k // 4][:, (k % 4) * 128:(k % 4 + 1) * 128]
                            nc.tensor.matmul(
                                c1_ps[p // 4][:, (p % 4) * 128:(p % 4 + 1) * 128],
                                w1_t[:, dy * 128:(dy + 1) * 128], rhs,
                                start=(dy == 0), stop=(dy == 4))

                    c1s = pa.tile([128, 1152], F16, tag="c1s")
                    for b in range(3):
                        w = 512 if b < 2 else 128
                        if b % 2 == 0:
                            nc.scalar.activation(
                                c1s[:, b * 512:b * 512 + w], c1_ps[b][:, :w],
                                mybir.ActivationFunctionType.Relu, bias=bc1_t[:, 0:1])
                        else:
                            nc.vector.tensor_scalar(
                                c1s[:, b * 512:b * 512 + w], c1_ps[b][:, :w],
                                bc1_t[:, 0:1], 0.0, op.add, op.max)

                    # --- conv2 (9 accumulating matmuls) + relu
                    c2_ps = psS.tile([128, 128], F32, tag="small", name="c2_ps")
                    for j2 in range(9):
                        nc.tensor.matmul(
                            c2_ps[:], w2_t[:, j2 * 128:(j2 + 1) * 128],
                            c1s[:, j2 * 128:(j2 + 1) * 128],
                            start=(j2 == 0), stop=(j2 == 8))
                    c2s = pa.tile([128, 128], F16, tag="c2s")
                    nc.scalar.activation(c2s[:], c2_ps[:],
                                         mybir.ActivationFunctionType.Relu,
                                         bias=bc2_t[:, 0:1])

                    # --- fc + relu -> hid[:, 256:512]
                    fc_ps = psS.tile([128, 256], F32, tag="small", name="fc_ps")
                    for h in range(2):
                        nc.tensor.matmul(fc_ps[:, h * 128:(h + 1) * 128],
                                         fc_t[:, h * 128:(h + 1) * 128], c2s[:],
                                         start=True, stop=True)
                        nc.scalar.activation(
                            hid[:, 256 + h * 128:256 + (h + 1) * 128],
                            fc_ps[:, h * 128:(h + 1) * 128],
                            mybir.ActivationFunctionType.Relu,
                            bias=bfc_t[:, h:h + 1])

                    # --- heads: out[20, rows] = headw.T @ hid
                    h_ps = psS.tile([20, 128], F32, tag="small", name="h_ps")
                    for kk in range(4):
                        nc.tensor.matmul(h_ps[:], hw_t[:, kk * 20:(kk + 1) * 20],
                                         hid[:, kk * 128:(kk + 1) * 128],
                                         start=(kk == 0), stop=(kk == 3))
                    h_sT = pa.tile([20, 128], F16, tag="hsT")
                    nc.vector.tensor_scalar(h_sT[:], h_ps[:], bhead_t[:, 0:1],
                                            None, op.add)
                    hT_ps = psS.tile([128, 20], F16, tag="small", name="hT_ps")
                    nc.tensor.transpose(hT_ps[:], h_sT[:], id_t[:20, :20])
                    out_s = pa.tile([128, 20], F32, tag="outs")
                    nc.vector.tensor_copy(out_s[:], hT_ps[:])
                    nc.sync.dma_start(out_d[t0 * 128:(t0 + 1) * 128, :], out_s[:])

    nc.compile()
    return nc


def _consts(inputs):
    f16 = np.float16
    c1w = np.asarray(inputs["conv1_w"], np.float32)      # [128, 22, 5, 5]
    w1c = np.zeros((110, 640), f16)
    for a in range(22):
        for dx in range(5):
            for dy in range(5):
                w1c[dx * 22 + a, dy * 128:(dy + 1) * 128] = (
                    c1w[:, a, dx, dy] / _DIV[a]).astype(f16)
    c2w = np.asarray(inputs["conv2_w"], np.float32)      # [128, 128, 3, 3]
    w2c = np.zeros((128, 1152), f16)
    for kx in range(3):
        for ky in range(3):
            j = kx * 3 + ky
            w2c[:, j * 128:(j + 1) * 128] = c2w[:, :, kx, ky].T.astype(f16)
    fcw = np.asarray(inputs["fc_w"], np.float32)         # [256, 128]
    fcwc = fcw.T.astype(f16)                             # [128, 256]
    sw = np.asarray(inputs["self_w"], np.float32)        # [256, 22]
    selfwc = (sw / _DIV[None, :]).T.astype(f16)          # [22, 256]
    hw = np.concatenate([np.asarray(inputs["actor0_w"]),
                         np.asarray(inputs["actor1_w"]),
                         np.asarray(inputs["value_w"])], axis=0)  # [20, 512]
    headwc = np.zeros((128, 80), f16)
    for kk in range(4):
        headwc[:, kk * 20:(kk + 1) * 20] = hw[:, kk * 128:(kk + 1) * 128].T.astype(f16)
    bhead = np.concatenate([np.asarray(inputs["actor0_b"]),
                            np.asarray(inputs["actor1_b"]),
                            np.asarray(inputs["value_b"])])[:, None].astype(np.float32)
    bfc = np.asarray(inputs["fc_b"], np.float32).reshape(2, 128).T.copy()
    bself = np.asarray(inputs["self_b"], np.float32).reshape(2, 128).T.copy()
    return {
        "w1c": w1c, "w2c": w2c, "fcwc": fcwc, "selfwc": selfwc,
        "headwc": headwc, "identc": np.eye(128, dtype=f16),
        "bc1": np.asarray(inputs["conv1_b"], np.float32)[:, None],
        "bc2": np.asarray(inputs["conv2_b"], np.float32)[:, None],
        "bfc": np.ascontiguousarray(bfc), "bself": np.ascontiguousarray(bself),
        "bhead": bhead,
    }


_NC = None


def _get_nc():
    global _NC
    if _NC is None:
        _NC = _build()
    return _NC


def _run(inputs, trace=False, tmpdir=None):
    nc = _get_nc()
    consts = _consts(inputs)
    obs = np.ascontiguousarray(
        np.asarray(inputs["observations"], np.int32).reshape(B, 3 * M))
    in_maps = []
    for c in range(N_CORES):
        m = dict(consts)
        m["obs"] = obs[c * B_C:(c + 1) * B_C]
        in_maps.append(m)
    res = bass_utils.run_bass_kernel_spmd(
        nc, in_maps, core_ids=list(range(N_CORES)), trace=trace, tmpdir=tmpdir)
    out = np.concatenate([res.results[c]["out"] for c in range(N_CORES)], axis=0)
    a0 = out[:, 0:9].astype(np.float32)
    a1 = out[:, 9:19].astype(np.float32)
    v = out[:, 19:20].astype(np.float32)
    return (a0, a1, v), res


def kernel(**inputs):
    (a0, a1, v), _ = _run(inputs)
    return a0, a1, v


# revision 8
# speedup vs baseline: 1.2175x; 1.2175x over previous
"""Trainium2 Bass kernel for nn_Policy_31104153158020.

Policy network: token parse -> scatter into [22,11,11] grid -> CNN trunk
(conv 5x5 s3 -> conv 3x3 -> fc) + self encoder on center cell -> 3 heads.

Strategy (pure data parallel over 8 cores, B=16384 -> 2048 rows/core):
- Parse tokens on DVE (bit ops) in groups of 4 row-tiles.
- Scatter per 128-row tile with GPSIMD local_scatter (two overlapping
  halves; HW processes indices in token order => last-write-wins,
  matching XLA scatter semantics exactly).
- conv1 as 45 strided PE transposes (im2col chunks [a,dx] x 5 dy) feeding
  45 fp16 matmuls; conv2/fc/self/heads as plain matmuls; normalization
  (1/max_vec) folded into conv1/self weights on the host.
"""
import os
import sys

sys.path.insert(0, "/opt/trn_rl_repo")

import numpy as np

import concourse.bass as bass
import concourse.bacc as bacc
import concourse.mybir as mybir
import concourse.tile as tile
from concourse import bass_utils

from concourse.ap import AP
from concourse.alu_op_type import AluOpType as op

N_CORES = 8
B = 16384
M = 200
B_C = B // N_CORES          # 2048 rows per core
TILES = B_C // 128          # 16 tiles of 128 rows
GROUP = 4                   # tiles parsed together
N_GROUPS = TILES // GROUP

NUM_LAYERS, OW, OH = 22, 11, 11
NCELL = NUM_LAYERS * OW * OH          # 2662
# scatter split: A covers cells [0,1332) (+dump 1332), B covers [1332,2662)
NE_A = 1334
NE_B = 1334
BOXW = 1332 + NE_B                    # 2666 box columns; box[:, c] = cell c

F16 = mybir.dt.float16
F32 = mybir.dt.float32
I16 = mybir.dt.int16
I32 = mybir.dt.int32

_MAX_VEC = np.maximum(np.array([9.0, 1.0, 1.0, 10.0, 3.0, 254.0, 1.0, 1.0, 235.0, 8.0,
                                9.0, 250.0, 29.0, 1.0, 1.0, 8.0, 1.0, 1.0, 6.0, 3.0,
                                1.0, 2.0], dtype=np.float32), 1.0)
_DIV = _MAX_VEC + 1e-8


def _build():
    nc = bacc.Bacc("TRN2", target_bir_lowering=False, debug=False,
                   num_devices=N_CORES)

    obs_d = nc.dram_tensor("obs", [B_C, 3 * M], I32, kind="ExternalInput").ap()
    w1_d = nc.dram_tensor("w1c", [110, 640], F16, kind="ExternalInput").ap()
    w2_d = nc.dram_tensor("w2c", [128, 1152], F16, kind="ExternalInput").ap()
    fc_d = nc.dram_tensor("fcwc", [128, 256], F16, kind="ExternalInput").ap()
    sw_d = nc.dram_tensor("selfwc", [22, 256], F16, kind="ExternalInput").ap()
    hw_d = nc.dram_tensor("headwc", [128, 80], F16, kind="ExternalInput").ap()
    id_d = nc.dram_tensor("identc", [128, 128], F16, kind="ExternalInput").ap()
    bc1_d = nc.dram_tensor("bc1", [128, 1], F32, kind="ExternalInput").ap()
    bc2_d = nc.dram_tensor("bc2", [128, 1], F32, kind="ExternalInput").ap()
    bfc_d = nc.dram_tensor("bfc", [128, 2], F32, kind="ExternalInput").ap()
    bself_d = nc.dram_tensor("bself", [128, 2], F32, kind="ExternalInput").ap()
    bhead_d = nc.dram_tensor("bhead", [20, 1], F32, kind="ExternalInput").ap()
    out_d = nc.dram_tensor("out", [20, B_C], F32, kind="ExternalOutput").ap()

    with tile.TileContext(nc) as tc:
        with (
            tc.tile_pool(name="const", bufs=1) as pc,
            tc.tile_pool(name="grp", bufs=2) as pg,
            tc.tile_pool(name="tok", bufs=2) as pt,
            tc.tile_pool(name="boxp", bufs=2) as pb,
            tc.tile_pool(name="imc", bufs=2) as pi,
            tc.tile_pool(name="act", bufs=2) as pa,
            tc.tile_pool(name="psT", bufs=2, space="PSUM") as psT,
            tc.tile_pool(name="psC", bufs=2, space="PSUM") as psC,
            tc.tile_pool(name="psS", bufs=3, space="PSUM") as psS,
        ):
            w1_t = pc.tile([110, 640], F16)
            w2_t = pc.tile([128, 1152], F16)
            fc_t = pc.tile([128, 256], F16)
            sw_t = pc.tile([22, 256], F16)
            hw_t = pc.tile([128, 80], F16)
            id_t = pc.tile([128, 128], F16)
            bc1_t = pc.tile([128, 1], F32)
            bc2_t = pc.tile([128, 1], F32)
            bfc_t = pc.tile([128, 2], F32)
            bself_t = pc.tile([128, 2], F32)
            bhead_t = pc.tile([20, 1], F32)
            nc.sync.dma_start(w1_t[:], w1_d)
            nc.sync.dma_start(w2_t[:], w2_d)
            nc.sync.dma_start(fc_t[:], fc_d)
            nc.sync.dma_start(sw_t[:], sw_d)
            nc.sync.dma_start(hw_t[:], hw_d)
            nc.sync.dma_start(id_t[:], id_d)
            nc.sync.dma_start(bc1_t[:], bc1_d)
            nc.sync.dma_start(bc2_t[:], bc2_d)
            nc.sync.dma_start(bfc_t[:], bfc_d)
            nc.sync.dma_start(bself_t[:], bself_d)
            nc.sync.dma_start(bhead_t[:], bhead_d)

            for g in range(N_GROUPS):
                FD = GROUP * M
                obs_g = pg.tile([128, GROUP * 3 * M], I32, tag="obs")
                for j in range(GROUP):
                    t0 = g * GROUP + j
                    nc.sync.dma_start(
                        obs_g[:, j * 3 * M:(j + 1) * 3 * M],
                        obs_d[t0 * 128:(t0 + 1) * 128, :],
                    )

                # token fields, strided int32 -> int16 casts
                c16 = pt.tile([128, FD], I16, tag="c16")
                a16 = pt.tile([128, FD], I16, tag="a16")
                vals = pt.tile([128, FD], F16, tag="vals")
                obs_v = obs_g[:]
                nc.vector.tensor_copy(c16[:], obs_v[:, 0::3])
                nc.vector.tensor_copy(a16[:], obs_v[:, 1::3])
                nc.vector.tensor_copy(vals[:], obs_v[:, 2::3])

                y16 = pt.tile([128, FD], I16, tag="y16")
                x16 = pt.tile([128, FD], I16, tag="x16")
                t1 = pt.tile([128, FD], I16, tag="t1")
                cell = pt.tile([128, FD], I16, tag="cell")
                m0 = pt.tile([128, FD], I16, tag="m0")
                m1 = pt.tile([128, FD], I16, tag="m1")
                idxA = pt.tile([128, FD], I16, tag="idxA")
                idxB = pt.tile([128, FD], I16, tag="idxB")

                nc.vector.tensor_scalar(y16[:], c16[:], 15, None, op.bitwise_and)
                nc.vector.tensor_scalar(x16[:], c16[:], 4, None, op.logical_shift_right)
                # cell = 242*x + 11*a + y  (im2col-friendly permutation:
                # chunk (p,dy) = single stride-11 run over (dx, a))
                nc.vector.tensor_scalar(t1[:], x16[:], 242, None, op.mult)
                nc.vector.tensor_scalar(cell[:], a16[:], 11, None, op.mult)
                nc.vector.tensor_tensor(cell[:], cell[:], t1[:], op.add)
                nc.vector.tensor_tensor(cell[:], cell[:], y16[:], op.add)
                # validity mask
                nc.vector.tensor_scalar(m0[:], x16[:], 11, None, op.is_lt)
                nc.vector.tensor_scalar(m1[:], y16[:], 11, None, op.is_lt)
                nc.vector.tensor_tensor(m0[:], m0[:], m1[:], op.mult)
                nc.vector.tensor_scalar(m1[:], a16[:], 22, None, op.is_lt)
                nc.vector.tensor_tensor(m0[:], m0[:], m1[:], op.mult)
                # invalid -> cell + 16384 (wraps to negative for large cells)
                nc.vector.tensor_scalar(m0[:], m0[:], -16384, 16384, op.mult, op.add)
                nc.vector.tensor_tensor(cell[:], cell[:], m0[:], op.add)
                # split indices for the two scatter halves
                nc.vector.tensor_scalar(idxA[:], cell[:], 1332, None, op.min)
                nc.vector.tensor_scalar(idxB[:], cell[:], 2664, 1332, op.min, op.subtract)

                # --- scatter + transposes per row-tile; conv at N=512 ---
                imc = pi.tile([110, 45 * 512], F16, tag="imc", name="imc")
                ctr_g = pa.tile([22, 512], F16, tag="ctr_g", name="ctr_g")
                for j in range(GROUP):
                    t0 = g * GROUP + j
                    sl = slice(j * M, (j + 1) * M)
                    box = pb.tile([128, BOXW], F16, tag="box", name="box")
                    nc.gpsimd.local_scatter(
                        box[:, 0:NE_A], vals[:, sl], idxA[:, sl],
                        channels=128, num_elems=NE_A, num_idxs=M)
                    nc.gpsimd.local_scatter(
                        box[:, 1332:1332 + NE_B], vals[:, sl], idxB[:, sl],
                        channels=128, num_elems=NE_B, num_idxs=M)

                    box_ap = box[:]
                    part_dim = list(box_ap.ap)[0]

                    # center cells (x=5,y=5) -> ctr_g columns
                    ctr_ps = psS.tile([22, 128], F16, tag="small", name="ctr_ps")
                    ctr_in = AP(box_ap.tensor, box_ap.offset + 1215,
                                [part_dim, [11, 22]])
                    nc.tensor.transpose(ctr_ps[:], ctr_in, id_t[:])
                    nc.vector.tensor_copy(ctr_g[:, j * 128:(j + 1) * 128], ctr_ps[:])

                    # 45 im2col chunk transposes, 8 per fp16 psum bank
                    psb = None
                    for k in range(45):
                        p, dy = divmod(k, 5)
                        ox, oy = divmod(p, 3)
                        if k % 8 == 0:
                            psb = psT.tile([110, 1024], F16, tag="T", name="psb")
                        chunk = AP(box_ap.tensor,
                                   box_ap.offset + 726 * ox + 3 * oy + dy,
                                   [part_dim, [11, 110]])
                        nc.tensor.transpose(
                            psb[:, (k % 8) * 128:(k % 8 + 1) * 128], chunk, id_t[:])
                        if k % 8 == 7 or k == 44:
                            b = k // 8
                            nk = k % 8 + 1
                            imct = imc[:]
                            dst = AP(imct.tensor, imct.offset + (b * 8) * 512 + j * 128,
                                     [list(imct.ap)[0], [512, nk], [1, 128]])
                            if (j + b) % 2 == 0:
                                nc.vector.tensor_copy(dst, psb[:, :nk * 128])
                            else:
                                nc.scalar.copy(dst, psb[:, :nk * 128])

                # --- self encoder (N=512)
                hidA = pa.tile([128, 512], F16, tag="hidA", name="hidA")
                hidB = pa.tile([128, 512], F16, tag="hidB", name="hidB")
                hidC = pa.tile([128, 512], F16, tag="hidC", name="hidC")
                hidD = pa.tile([128, 512], F16, tag="hidD", name="hidD")
                for h, hout in ((0, hidA), (1, hidB)):
                    sf_ps = psS.tile([128, 512], F32, tag="small", name="sf_ps")
                    nc.tensor.matmul(sf_ps[:], sw_t[:, h * 128:(h + 1) * 128],
                                     ctr_g[:], start=True, stop=True)
                    nc.scalar.activation(hout[:], sf_ps[:],
                                         mybir.ActivationFunctionType.Relu,
                                         bias=bself_t[:, h:h + 1])

                # --- conv1 (45 matmuls at N=512) + relu
                c1s = pa.tile([128, 9 * 512], F16, tag="c1s", name="c1s")
                for p in range(9):
                    c1_ps = psC.tile([128, 512], F32, tag="c1", name="c1_ps")
                    for dy in range(5):
                        k = p * 5 + dy
                        nc.tensor.matmul(
                            c1_ps[:], w1_t[:, dy * 128:(dy + 1) * 128],
                            imc[:, k * 512:(k + 1) * 512],
                            start=(dy == 0), stop=(dy == 4))
                    if p % 2 == 0:
                        nc.scalar.activation(
                            c1s[:, p * 512:(p + 1) * 512], c1_ps[:],
                            mybir.ActivationFunctionType.Relu, bias=bc1_t[:, 0:1])
                    else:
                        nc.vector.tensor_scalar(
                            c1s[:, p * 512:(p + 1) * 512], c1_ps[:],
                            bc1_t[:, 0:1], 0.0, op.add, op.max)

                # --- conv2 (9 accumulating matmuls at N=512) + relu
                c2_ps = psS.tile([128, 512], F32, tag="small", name="c2_ps")
                for j2 in range(9):
                    nc.tensor.matmul(c2_ps[:], w2_t[:, j2 * 128:(j2 + 1) * 128],
                                     c1s[:, j2 * 512:(j2 + 1) * 512],
                                     start=(j2 == 0), stop=(j2 == 8))
                c2s = pa.tile([128, 512], F16, tag="c2s", name="c2s")
                nc.vector.tensor_scalar(c2s[:], c2_ps[:], bc2_t[:, 0:1], 0.0,
                                        op.add, op.max)

                # --- fc + relu
                for h, hout in ((0, hidC), (1, hidD)):
                    fc_ps = psS.tile([128, 512], F32, tag="small", name="fc_ps")
                    nc.tensor.matmul(fc_ps[:], fc_t[:, h * 128:(h + 1) * 128],
                                     c2s[:], start=True, stop=True)
                    nc.scalar.activation(hout[:], fc_ps[:],
                                         mybir.ActivationFunctionType.Relu,
                                         bias=bfc_t[:, h:h + 1])

                # --- heads (N=512) -> out[20, rows]
                h_ps = psS.tile([20, 512], F32, tag="small", name="h_ps")
                for kk, hin in enumerate((hidA, hidB, hidC, hidD)):
                    nc.tensor.matmul(h_ps[:], hw_t[:, kk * 20:(kk + 1) * 20],
                                     hin[:], start=(kk == 0), stop=(kk == 3))
                outT = pa.tile([20, 512], F32, tag="outT", name="outT")
                nc.vector.tensor_scalar(outT[:], h_ps[:], bhead_t[:, 0:1],
                                        None, op.add)
                nc.sync.dma_start(out_d[:, g * 512:(g + 1) * 512], outT[:])

    nc.compile()
    return nc


def _consts(inputs):
    f16 = np.float16
    c1w = np.asarray(inputs["conv1_w"], np.float32)      # [128, 22, 5, 5]
    w1c = np.zeros((110, 640), f16)
    for a in range(22):
        for dx in range(5):
            for dy in range(5):
                w1c[dx * 22 + a, dy * 128:(dy + 1) * 128] = (
                    c1w[:, a, dx, dy] / _DIV[a]).astype(f16)
    c2w = np.asarray(inputs["conv2_w"], np.float32)      # [128, 128, 3, 3]
    w2c = np.zeros((128, 1152), f16)
    for kx in range(3):
        for ky in range(3):
            j = kx * 3 + ky
            w2c[:, j * 128:(j + 1) * 128] = c2w[:, :, kx, ky].T.astype(f16)
    fcw = np.asarray(inputs["fc_w"], np.float32)         # [256, 128]
    fcwc = fcw.T.astype(f16)                             # [128, 256]
    sw = np.asarray(inputs["self_w"], np.float32)        # [256, 22]
    selfwc = (sw / _DIV[None, :]).T.astype(f16)          # [22, 256]
    hw = np.concatenate([np.asarray(inputs["actor0_w"]),
                         np.asarray(inputs["actor1_w"]),
                         np.asarray(inputs["value_w"])], axis=0)  # [20, 512]
    headwc = np.zeros((128, 80), f16)
    for kk in range(4):
        headwc[:, kk * 20:(kk + 1) * 20] = hw[:, kk * 128:(kk + 1) * 128].T.astype(f16)
    bhead = np.concatenate([np.asarray(inputs["actor0_b"]),
                            np.asarray(inputs["actor1_b"]),
                            np.asarray(inputs["value_b"])])[:, None].astype(np.float32)
    bfc = np.asarray(inputs["fc_b"], np.float32).reshape(2, 128).T.copy()
    bself = np.asarray(inputs["self_b"], np.float32).reshape(2, 128).T.copy()
    return {
        "w1c": w1c, "w2c": w2c, "fcwc": fcwc, "selfwc": selfwc,
        "headwc": headwc, "identc": np.eye(128, dtype=f16),
        "bc1": np.asarray(inputs["conv1_b"], np.float32)[:, None],
        "bc2": np.asarray(inputs["conv2_b"], np.float32)[:, None],
        "bfc": np.ascontiguousarray(bfc), "bself": np.ascontiguousarray(bself),
        "bhead": bhead,
    }


_NC = None


def _get_nc():
    global _NC
    if _NC is None:
        _NC = _build()
    return _NC


def _run(inputs, trace=False, tmpdir=None):
    nc = _get_nc()
    consts = _consts(inputs)
    obs = np.ascontiguousarray(
        np.asarray(inputs["observations"], np.int32).reshape(B, 3 * M))
    in_maps = []
    for c in range(N_CORES):
        m = dict(consts)
        m["obs"] = obs[c * B_C:(c + 1) * B_C]
        in_maps.append(m)
    res = bass_utils.run_bass_kernel_spmd(
        nc, in_maps, core_ids=list(range(N_CORES)), trace=trace, tmpdir=tmpdir)
    out = np.concatenate([res.results[c]["out"].T for c in range(N_CORES)], axis=0)
    a0 = np.ascontiguousarray(out[:, 0:9], dtype=np.float32)
    a1 = np.ascontiguousarray(out[:, 9:19], dtype=np.float32)
    v = np.ascontiguousarray(out[:, 19:20], dtype=np.float32)
    return (a0, a1, v), res


def kernel(**inputs):
    (a0, a1, v), _ = _run(inputs)
    return a0, a1, v


# revision 9
# speedup vs baseline: 1.3325x; 1.0945x over previous
"""Trainium2 Bass kernel for nn_Policy_31104153158020.

Policy network: token parse -> scatter into [22,11,11] grid -> CNN trunk
(conv 5x5 s3 -> conv 3x3 -> fc) + self encoder on center cell -> 3 heads.

Strategy (pure data parallel over 8 cores, B=16384 -> 2048 rows/core):
- Parse tokens on DVE (bit ops) in groups of 4 row-tiles.
- Scatter per 128-row tile with GPSIMD local_scatter (two overlapping
  halves; HW processes indices in token order => last-write-wins,
  matching XLA scatter semantics exactly).
- conv1 as 45 strided PE transposes (im2col chunks [a,dx] x 5 dy) feeding
  45 fp16 matmuls; conv2/fc/self/heads as plain matmuls; normalization
  (1/max_vec) folded into conv1/self weights on the host.
"""
import os
import sys

sys.path.insert(0, "/opt/trn_rl_repo")

import numpy as np

import concourse.bass as bass
import concourse.bacc as bacc
import concourse.mybir as mybir
import concourse.tile as tile
from concourse import bass_utils

from concourse.ap import AP
from concourse.alu_op_type import AluOpType as op

N_CORES = 8
B = 16384
M = 200
B_C = B // N_CORES          # 2048 rows per core
TILES = B_C // 128          # 16 tiles of 128 rows
GROUP = 4                   # tiles parsed together
N_GROUPS = TILES // GROUP

NUM_LAYERS, OW, OH = 22, 11, 11
NCELL = NUM_LAYERS * OW * OH          # 2662
# scatter split: A covers cells [0,1332) (+dump 1332), B covers [1332,2662)
NE_A = 1334
NE_B = 1334
BOXW = 2880                           # >= 1462 + 11*128; box[:, c] = cell c

F16 = mybir.dt.float16
F32 = mybir.dt.float32
I16 = mybir.dt.int16
I32 = mybir.dt.int32

_MAX_VEC = np.maximum(np.array([9.0, 1.0, 1.0, 10.0, 3.0, 254.0, 1.0, 1.0, 235.0, 8.0,
                                9.0, 250.0, 29.0, 1.0, 1.0, 8.0, 1.0, 1.0, 6.0, 3.0,
                                1.0, 2.0], dtype=np.float32), 1.0)
_DIV = _MAX_VEC + 1e-8


def _build():
    nc = bacc.Bacc("TRN2", target_bir_lowering=False, debug=False,
                   num_devices=N_CORES)

    obs_d = nc.dram_tensor("obs", [B_C, 3 * M], I32, kind="ExternalInput").ap()
    w1_d = nc.dram_tensor("w1c", [110, 640], F16, kind="ExternalInput").ap()
    w2_d = nc.dram_tensor("w2c", [128, 1152], F16, kind="ExternalInput").ap()
    fc_d = nc.dram_tensor("fcwc", [128, 256], F16, kind="ExternalInput").ap()
    sw_d = nc.dram_tensor("selfwc", [22, 256], F16, kind="ExternalInput").ap()
    hw_d = nc.dram_tensor("headwc", [128, 80], F16, kind="ExternalInput").ap()
    id_d = nc.dram_tensor("identc", [128, 128], F16, kind="ExternalInput").ap()
    bc1_d = nc.dram_tensor("bc1", [128, 1], F32, kind="ExternalInput").ap()
    bc2_d = nc.dram_tensor("bc2", [128, 1], F32, kind="ExternalInput").ap()
    bfc_d = nc.dram_tensor("bfc", [128, 2], F32, kind="ExternalInput").ap()
    bself_d = nc.dram_tensor("bself", [128, 2], F32, kind="ExternalInput").ap()
    bhead_d = nc.dram_tensor("bhead", [20, 1], F32, kind="ExternalInput").ap()
    out_d = nc.dram_tensor("out", [20, B_C], F32, kind="ExternalOutput").ap()

    with tile.TileContext(nc) as tc:
        with (
            tc.tile_pool(name="const", bufs=1) as pc,
            tc.tile_pool(name="grp", bufs=2) as pg,
            tc.tile_pool(name="tok", bufs=2) as pt,
            tc.tile_pool(name="boxp", bufs=2) as pb,
            tc.tile_pool(name="imc", bufs=2) as pi,
            tc.tile_pool(name="act", bufs=2) as pa,
            tc.tile_pool(name="psT", bufs=2, space="PSUM") as psT,
            tc.tile_pool(name="psC", bufs=2, space="PSUM") as psC,
            tc.tile_pool(name="psS", bufs=3, space="PSUM") as psS,
        ):
            w1_t = pc.tile([110, 640], F16)
            w2_t = pc.tile([128, 1152], F16)
            fc_t = pc.tile([128, 256], F16)
            sw_t = pc.tile([22, 256], F16)
            hw_t = pc.tile([128, 80], F16)
            id_t = pc.tile([128, 128], F16)
            bc1_t = pc.tile([128, 1], F32)
            bc2_t = pc.tile([128, 1], F32)
            bfc_t = pc.tile([128, 2], F32)
            bself_t = pc.tile([128, 2], F32)
            bhead_t = pc.tile([20, 1], F32)
            nc.sync.dma_start(w1_t[:], w1_d)
            nc.sync.dma_start(w2_t[:], w2_d)
            nc.sync.dma_start(fc_t[:], fc_d)
            nc.sync.dma_start(sw_t[:], sw_d)
            nc.sync.dma_start(hw_t[:], hw_d)
            nc.sync.dma_start(id_t[:], id_d)
            nc.sync.dma_start(bc1_t[:], bc1_d)
            nc.sync.dma_start(bc2_t[:], bc2_d)
            nc.sync.dma_start(bfc_t[:], bfc_d)
            nc.sync.dma_start(bself_t[:], bself_d)
            nc.sync.dma_start(bhead_t[:], bhead_d)

            for g in range(N_GROUPS):
                FD = GROUP * M
                obs_g = pg.tile([128, GROUP * 3 * M], I32, tag="obs")
                for j in range(GROUP):
                    t0 = g * GROUP + j
                    nc.sync.dma_start(
                        obs_g[:, j * 3 * M:(j + 1) * 3 * M],
                        obs_d[t0 * 128:(t0 + 1) * 128, :],
                    )

                # token fields, strided int32 -> int16 casts
                c16 = pt.tile([128, FD], I16, tag="c16")
                a16 = pt.tile([128, FD], I16, tag="a16")
                vals = pt.tile([128, FD], F16, tag="vals")
                obs_v = obs_g[:]
                nc.vector.tensor_copy(c16[:], obs_v[:, 0::3])
                nc.vector.tensor_copy(a16[:], obs_v[:, 1::3])
                nc.vector.tensor_copy(vals[:], obs_v[:, 2::3])

                y16 = pt.tile([128, FD], I16, tag="y16")
                x16 = pt.tile([128, FD], I16, tag="x16")
                t1 = pt.tile([128, FD], I16, tag="t1")
                cell = pt.tile([128, FD], I16, tag="cell")
                m0 = pt.tile([128, FD], I16, tag="m0")
                m1 = pt.tile([128, FD], I16, tag="m1")
                idxA = pt.tile([128, FD], I16, tag="idxA")
                idxB = pt.tile([128, FD], I16, tag="idxB")

                nc.vector.tensor_scalar(y16[:], c16[:], 15, None, op.bitwise_and)
                nc.vector.tensor_scalar(x16[:], c16[:], 4, None, op.logical_shift_right)
                # cell = 242*x + 11*a + y  (im2col-friendly permutation:
                # chunk (p,dy) = single stride-11 run over (dx, a))
                nc.vector.tensor_scalar(t1[:], x16[:], 242, None, op.mult)
                nc.vector.tensor_scalar(cell[:], a16[:], 11, None, op.mult)
                nc.vector.tensor_tensor(cell[:], cell[:], t1[:], op.add)
                nc.vector.tensor_tensor(cell[:], cell[:], y16[:], op.add)
                # validity mask
                nc.vector.tensor_scalar(m0[:], x16[:], 11, None, op.is_lt)
                nc.vector.tensor_scalar(m1[:], y16[:], 11, None, op.is_lt)
                nc.vector.tensor_tensor(m0[:], m0[:], m1[:], op.mult)
                nc.vector.tensor_scalar(m1[:], a16[:], 22, None, op.is_lt)
                nc.vector.tensor_tensor(m0[:], m0[:], m1[:], op.mult)
                # invalid -> cell + 16384 (wraps to negative for large cells)
                nc.vector.tensor_scalar(m0[:], m0[:], -16384, 16384, op.mult, op.add)
                nc.vector.tensor_tensor(cell[:], cell[:], m0[:], op.add)
                # split indices for the two scatter halves
                nc.vector.tensor_scalar(idxA[:], cell[:], 1332, None, op.min)
                nc.vector.tensor_scalar(idxB[:], cell[:], 2664, 1332, op.min, op.subtract)

                # --- scatter + transposes per row-tile; conv at N=512 ---
                imc = pi.tile([110, 45 * 512], F16, tag="imc", name="imc")
                ctr_g = pa.tile([22, 512], F16, tag="ctr_g", name="ctr_g")
                for j in range(GROUP):
                    t0 = g * GROUP + j
                    sl = slice(j * M, (j + 1) * M)
                    box = pb.tile([128, BOXW], F16, tag="box", name="box")
                    nc.gpsimd.local_scatter(
                        box[:, 0:NE_A], vals[:, sl], idxA[:, sl],
                        channels=128, num_elems=NE_A, num_idxs=M)
                    nc.gpsimd.local_scatter(
                        box[:, 1332:1332 + NE_B], vals[:, sl], idxB[:, sl],
                        channels=128, num_elems=NE_B, num_idxs=M)

                    box_ap = box[:]
                    part_dim = list(box_ap.ap)[0]

                    # center cells (x=5,y=5) -> ctr_g columns
                    ctr_ps = psS.tile([22, 128], F32, tag="small", name="ctr_ps")
                    ctr_in = AP(box_ap.tensor, box_ap.offset + 1215,
                                [part_dim, [11, 22]])
                    nc.tensor.matmul(ctr_ps[:], ctr_in, id_t[:], start=True, stop=True)
                    nc.vector.tensor_copy(ctr_g[:, j * 128:(j + 1) * 128], ctr_ps[:])

                    # 45 im2col chunk transposes, 8 per fp16 psum bank
                    psb = None
                    for k in range(45):
                        p, dy = divmod(k, 5)
                        ox, oy = divmod(p, 3)
                        if k % 4 == 0:
                            psb = psT.tile([128, 512], F32, tag="T", name="psb")
                        # transpose as a REGULAR matmul (chunk.T @ I) so the
                        # PE HAM clock-boost engages (transpose-mode never
                        # warms); M padded to 128 for fast weight load.
                        chunk = AP(box_ap.tensor,
                                   box_ap.offset + 726 * ox + 3 * oy + dy,
                                   [part_dim, [11, 128]])
                        nc.tensor.matmul(
                            psb[:, (k % 4) * 128:(k % 4 + 1) * 128], chunk,
                            id_t[:], start=True, stop=True)
                        if k % 4 == 3 or k == 44:
                            b = k // 4
                            nk = k % 4 + 1
                            imct = imc[:]
                            dst = AP(imct.tensor, imct.offset + (b * 4) * 512 + j * 128,
                                     [list(imct.ap)[0], [512, nk], [1, 128]])
                            if (j + b) % 2 == 0:
                                nc.vector.tensor_copy(dst, psb[:110, :nk * 128])
                            else:
                                nc.scalar.copy(dst, psb[:110, :nk * 128])

                # --- self encoder (N=512)
                hidA = pa.tile([128, 512], F16, tag="hidA", name="hidA")
                hidB = pa.tile([128, 512], F16, tag="hidB", name="hidB")
                hidC = pa.tile([128, 512], F16, tag="hidC", name="hidC")
                hidD = pa.tile([128, 512], F16, tag="hidD", name="hidD")
                for h, hout in ((0, hidA), (1, hidB)):
                    sf_ps = psS.tile([128, 512], F32, tag="small", name="sf_ps")
                    nc.tensor.matmul(sf_ps[:], sw_t[:, h * 128:(h + 1) * 128],
                                     ctr_g[:], start=True, stop=True)
                    nc.scalar.activation(hout[:], sf_ps[:],
                                         mybir.ActivationFunctionType.Relu,
                                         bias=bself_t[:, h:h + 1])

                # --- conv1 (45 matmuls at N=512) + relu
                c1s = pa.tile([128, 9 * 512], F16, tag="c1s", name="c1s")
                for p in range(9):
                    c1_ps = psC.tile([128, 512], F32, tag="c1", name="c1_ps")
                    for dy in range(5):
                        k = p * 5 + dy
                        nc.tensor.matmul(
                            c1_ps[:], w1_t[:, dy * 128:(dy + 1) * 128],
                            imc[:, k * 512:(k + 1) * 512],
                            start=(dy == 0), stop=(dy == 4))
                    if p % 2 == 0:
                        nc.scalar.activation(
                            c1s[:, p * 512:(p + 1) * 512], c1_ps[:],
                            mybir.ActivationFunctionType.Relu, bias=bc1_t[:, 0:1])
                    else:
                        nc.vector.tensor_scalar(
                            c1s[:, p * 512:(p + 1) * 512], c1_ps[:],
                            bc1_t[:, 0:1], 0.0, op.add, op.max)

                # --- conv2 (9 accumulating matmuls at N=512) + relu
                c2_ps = psS.tile([128, 512], F32, tag="small", name="c2_ps")
                for j2 in range(9):
                    nc.tensor.matmul(c2_ps[:], w2_t[:, j2 * 128:(j2 + 1) * 128],
                                     c1s[:, j2 * 512:(j2 + 1) * 512],
                                     start=(j2 == 0), stop=(j2 == 8))
                c2s = pa.tile([128, 512], F16, tag="c2s", name="c2s")
                nc.vector.tensor_scalar(c2s[:], c2_ps[:], bc2_t[:, 0:1], 0.0,
                                        op.add, op.max)

                # --- fc + relu
                for h, hout in ((0, hidC), (1, hidD)):
                    fc_ps = psS.tile([128, 512], F32, tag="small", name="fc_ps")
                    nc.tensor.matmul(fc_ps[:], fc_t[:, h * 128:(h + 1) * 128],
                                     c2s[:], start=True, stop=True)
                    nc.scalar.activation(hout[:], fc_ps[:],
                                         mybir.ActivationFunctionType.Relu,
                                         bias=bfc_t[:, h:h + 1])

                # --- heads (N=512) -> out[20, rows]
                h_ps = psS.tile([20, 512], F32, tag="small", name="h_ps")
                for kk, hin in enumerate((hidA, hidB, hidC, hidD)):
                    nc.tensor.matmul(h_ps[:], hw_t[:, kk * 20:(kk + 1) * 20],
                                     hin[:], start=(kk == 0), stop=(kk == 3))
                outT = pa.tile([20, 512], F32, tag="outT", name="outT")
                nc.vector.tensor_scalar(outT[:], h_ps[:], bhead_t[:, 0:1],
                                        None, op.add)
                nc.sync.dma_start(out_d[:, g * 512:(g + 1) * 512], outT[:])

    nc.compile()
    return nc


def _consts(inputs):
    f16 = np.float16
    c1w = np.asarray(inputs["conv1_w"], np.float32)      # [128, 22, 5, 5]
    w1c = np.zeros((110, 640), f16)
    for a in range(22):
        for dx in range(5):
            for dy in range(5):
                w1c[dx * 22 + a, dy * 128:(dy + 1) * 128] = (
                    c1w[:, a, dx, dy] / _DIV[a]).astype(f16)
    c2w = np.asarray(inputs["conv2_w"], np.float32)      # [128, 128, 3, 3]
    w2c = np.zeros((128, 1152), f16)
    for kx in range(3):
        for ky in range(3):
            j = kx * 3 + ky
            w2c[:, j * 128:(j + 1) * 128] = c2w[:, :, kx, ky].T.astype(f16)
    fcw = np.asarray(inputs["fc_w"], np.float32)         # [256, 128]
    fcwc = fcw.T.astype(f16)                             # [128, 256]
    sw = np.asarray(inputs["self_w"], np.float32)        # [256, 22]
    selfwc = (sw / _DIV[None, :]).T.astype(f16)          # [22, 256]
    hw = np.concatenate([np.asarray(inputs["actor0_w"]),
                         np.asarray(inputs["actor1_w"]),
                         np.asarray(inputs["value_w"])], axis=0)  # [20, 512]
    headwc = np.zeros((128, 80), f16)
    for kk in range(4):
        headwc[:, kk * 20:(kk + 1) * 20] = hw[:, kk * 128:(kk + 1) * 128].T.astype(f16)
    bhead = np.concatenate([np.asarray(inputs["actor0_b"]),
                            np.asarray(inputs["actor1_b"]),
                            np.asarray(inputs["value_b"])])[:, None].astype(np.float32)
    bfc = np.asarray(inputs["fc_b"], np.float32).reshape(2, 128).T.copy()
    bself = np.asarray(inputs["self_b"], np.float32).reshape(2, 128).T.copy()
    return {
        "w1c": w1c, "w2c": w2c, "fcwc": fcwc, "selfwc": selfwc,
        "headwc": headwc, "identc": np.eye(128, dtype=f16),
        "bc1": np.asarray(inputs["conv1_b"], np.float32)[:, None],
        "bc2": np.asarray(inputs["conv2_b"], np.float32)[:, None],
        "bfc": np.ascontiguousarray(bfc), "bself": np.ascontiguousarray(bself),
        "bhead": bhead,
    }


_NC = None


def _get_nc():
    global _NC
    if _NC is None:
        _NC = _build()
    return _NC


def _run(inputs, trace=False, tmpdir=None):
    nc = _get_nc()
    consts = _consts(inputs)
    obs = np.ascontiguousarray(
        np.asarray(inputs["observations"], np.int32).reshape(B, 3 * M))
    in_maps = []
    for c in range(N_CORES):
        m = dict(consts)
        m["obs"] = obs[c * B_C:(c + 1) * B_C]
        in_maps.append(m)
    res = bass_utils.run_bass_kernel_spmd(
        nc, in_maps, core_ids=list(range(N_CORES)), trace=trace, tmpdir=tmpdir)
    out = np.concatenate([res.results[c]["out"].T for c in range(N_CORES)], axis=0)
    a0 = np.ascontiguousarray(out[:, 0:9], dtype=np.float32)
    a1 = np.ascontiguousarray(out[:, 9:19], dtype=np.float32)
    v = np.ascontiguousarray(out[:, 19:20], dtype=np.float32)
    return (a0, a1, v), res


def kernel(**inputs):
    (a0, a1, v), _ = _run(inputs)
    return a0, a1, v


# revision 10
# speedup vs baseline: 1.3523x; 1.0149x over previous
"""Trainium2 Bass kernel for nn_Policy_31104153158020.

Policy network: token parse -> scatter into [22,11,11] grid -> CNN trunk
(conv 5x5 s3 -> conv 3x3 -> fc) + self encoder on center cell -> 3 heads.

Strategy (pure data parallel over 8 cores, B=16384 -> 2048 rows/core):
- Parse tokens on DVE (bit ops) in groups of 4 row-tiles.
- Scatter per 128-row tile with GPSIMD local_scatter (two overlapping
  halves; HW processes indices in token order => last-write-wins,
  matching XLA scatter semantics exactly).
- conv1 as 45 strided PE transposes (im2col chunks [a,dx] x 5 dy) feeding
  45 fp16 matmuls; conv2/fc/self/heads as plain matmuls; normalization
  (1/max_vec) folded into conv1/self weights on the host.
"""
import os
import sys

sys.path.insert(0, "/opt/trn_rl_repo")

import numpy as np

import concourse.bass as bass
import concourse.bacc as bacc
import concourse.mybir as mybir
import concourse.tile as tile
from concourse import bass_utils

from concourse.ap import AP
from concourse.alu_op_type import AluOpType as op

N_CORES = 8
B = 16384
M = 200
B_C = B // N_CORES          # 2048 rows per core
TILES = B_C // 128          # 16 tiles of 128 rows
GROUP = 4                   # tiles parsed together
N_GROUPS = TILES // GROUP

NUM_LAYERS, OW, OH = 22, 11, 11
NCELL = NUM_LAYERS * OW * OH          # 2662
# scatter split: A covers cells [0,1332) (+dump 1332), B covers [1332,2662)
NE_A = 1334
NE_B = 1334
BOXW = 2880                           # >= 1462 + 11*128; box[:, c] = cell c

F16 = mybir.dt.float16
F32 = mybir.dt.float32
I16 = mybir.dt.int16
I32 = mybir.dt.int32

_MAX_VEC = np.maximum(np.array([9.0, 1.0, 1.0, 10.0, 3.0, 254.0, 1.0, 1.0, 235.0, 8.0,
                                9.0, 250.0, 29.0, 1.0, 1.0, 8.0, 1.0, 1.0, 6.0, 3.0,
                                1.0, 2.0], dtype=np.float32), 1.0)
_DIV = _MAX_VEC + 1e-8


def _build():
    nc = bacc.Bacc("TRN2", target_bir_lowering=False, debug=False,
                   num_devices=N_CORES)

    obs_d = nc.dram_tensor("obs", [B_C, 3 * M], I32, kind="ExternalInput").ap()
    w1_d = nc.dram_tensor("w1c", [110, 640], F16, kind="ExternalInput").ap()
    w2_d = nc.dram_tensor("w2c", [128, 1152], F16, kind="ExternalInput").ap()
    fc_d = nc.dram_tensor("fcwc", [128, 256], F16, kind="ExternalInput").ap()
    sw_d = nc.dram_tensor("selfwc", [22, 256], F16, kind="ExternalInput").ap()
    hw_d = nc.dram_tensor("headwc", [128, 80], F16, kind="ExternalInput").ap()
    id_d = nc.dram_tensor("identc", [128, 128], F16, kind="ExternalInput").ap()
    bc1_d = nc.dram_tensor("bc1", [128, 1], F32, kind="ExternalInput").ap()
    bc2_d = nc.dram_tensor("bc2", [128, 1], F32, kind="ExternalInput").ap()
    bfc_d = nc.dram_tensor("bfc", [128, 2], F32, kind="ExternalInput").ap()
    bself_d = nc.dram_tensor("bself", [128, 2], F32, kind="ExternalInput").ap()
    bhead_d = nc.dram_tensor("bhead", [20, 1], F32, kind="ExternalInput").ap()
    out_d = nc.dram_tensor("out", [20, B_C], F32, kind="ExternalOutput").ap()

    with tile.TileContext(nc) as tc:
        with (
            tc.tile_pool(name="const", bufs=1) as pc,
            tc.tile_pool(name="grp", bufs=2) as pg,
            tc.tile_pool(name="tok", bufs=2) as pt,
            tc.tile_pool(name="boxp", bufs=2) as pb,
            tc.tile_pool(name="imc", bufs=2) as pi,
            tc.tile_pool(name="act", bufs=2) as pa,
            tc.tile_pool(name="psT", bufs=2, space="PSUM") as psT,
            tc.tile_pool(name="psC", bufs=2, space="PSUM") as psC,
            tc.tile_pool(name="psS", bufs=3, space="PSUM") as psS,
        ):
            w1_t = pc.tile([110, 640], F16)
            w2_t = pc.tile([128, 1152], F16)
            fc_t = pc.tile([128, 256], F16)
            sw_t = pc.tile([22, 256], F16)
            hw_t = pc.tile([128, 80], F16)
            id_t = pc.tile([128, 128], F16)
            bc1_t = pc.tile([128, 1], F32)
            bc2_t = pc.tile([128, 1], F32)
            bfc_t = pc.tile([128, 2], F32)
            bself_t = pc.tile([128, 2], F32)
            bhead_t = pc.tile([20, 1], F32)
            nc.sync.dma_start(w1_t[:], w1_d)
            nc.sync.dma_start(w2_t[:], w2_d)
            nc.sync.dma_start(fc_t[:], fc_d)
            nc.sync.dma_start(sw_t[:], sw_d)
            nc.sync.dma_start(hw_t[:], hw_d)
            nc.sync.dma_start(id_t[:], id_d)
            nc.sync.dma_start(bc1_t[:], bc1_d)
            nc.sync.dma_start(bc2_t[:], bc2_d)
            nc.sync.dma_start(bfc_t[:], bfc_d)
            nc.sync.dma_start(bself_t[:], bself_d)
            nc.sync.dma_start(bhead_t[:], bhead_d)

            for g in range(N_GROUPS):
                FD = GROUP * M
                obs_g = pg.tile([128, GROUP * 3 * M], I32, tag="obs")
                for j in range(GROUP):
                    t0 = g * GROUP + j
                    nc.sync.dma_start(
                        obs_g[:, j * 3 * M:(j + 1) * 3 * M],
                        obs_d[t0 * 128:(t0 + 1) * 128, :],
                    )

                # token fields, strided int32 -> int16 casts
                c16 = pt.tile([128, FD], I16, tag="c16")
                a16 = pt.tile([128, FD], I16, tag="a16")
                vals = pt.tile([128, FD], F16, tag="vals")
                obs_v = obs_g[:]
                nc.vector.tensor_copy(c16[:], obs_v[:, 0::3])
                nc.vector.tensor_copy(a16[:], obs_v[:, 1::3])
                nc.vector.tensor_copy(vals[:], obs_v[:, 2::3])

                y16 = pt.tile([128, FD], I16, tag="y16")
                x16 = pt.tile([128, FD], I16, tag="x16")
                t1 = pt.tile([128, FD], I16, tag="t1")
                cell = pt.tile([128, FD], I16, tag="cell")
                m0 = pt.tile([128, FD], I16, tag="m0")
                m1 = pt.tile([128, FD], I16, tag="m1")
                idxA = pt.tile([128, FD], I16, tag="idxA")
                idxB = pt.tile([128, FD], I16, tag="idxB")

                nc.vector.tensor_scalar(y16[:], c16[:], 15, None, op.bitwise_and)
                nc.vector.tensor_scalar(x16[:], c16[:], 4, None, op.logical_shift_right)
                # cell = 242*y + 22*x + a  (im2col permutation: chunk
                # (p,dy) = one CONTIGUOUS 110-run over (dx, a) -> FWL)
                nc.vector.tensor_scalar(t1[:], y16[:], 242, None, op.mult)
                nc.vector.tensor_scalar(cell[:], x16[:], 22, None, op.mult)
                nc.vector.tensor_tensor(cell[:], cell[:], t1[:], op.add)
                nc.vector.tensor_tensor(cell[:], cell[:], a16[:], op.add)
                # validity mask
                nc.vector.tensor_scalar(m0[:], x16[:], 11, None, op.is_lt)
                nc.vector.tensor_scalar(m1[:], y16[:], 11, None, op.is_lt)
                nc.vector.tensor_tensor(m0[:], m0[:], m1[:], op.mult)
                nc.vector.tensor_scalar(m1[:], a16[:], 22, None, op.is_lt)
                nc.vector.tensor_tensor(m0[:], m0[:], m1[:], op.mult)
                # invalid -> cell + 16384 (wraps to negative for large cells)
                nc.vector.tensor_scalar(m0[:], m0[:], -16384, 16384, op.mult, op.add)
                nc.vector.tensor_tensor(cell[:], cell[:], m0[:], op.add)
                # split indices for the two scatter halves
                nc.vector.tensor_scalar(idxA[:], cell[:], 1332, None, op.min)
                nc.vector.tensor_scalar(idxB[:], cell[:], 2664, 1332, op.min, op.subtract)

                # --- scatter + transposes per row-tile; conv at N=512 ---
                imc = pi.tile([110, 45 * 512], F16, tag="imc", name="imc")
                ctr_g = pa.tile([22, 512], F16, tag="ctr_g", name="ctr_g")
                for j in range(GROUP):
                    t0 = g * GROUP + j
                    sl = slice(j * M, (j + 1) * M)
                    box = pb.tile([128, BOXW], F16, tag="box", name="box")
                    nc.gpsimd.local_scatter(
                        box[:, 0:NE_A], vals[:, sl], idxA[:, sl],
                        channels=128, num_elems=NE_A, num_idxs=M)
                    nc.gpsimd.local_scatter(
                        box[:, 1332:1332 + NE_B], vals[:, sl], idxB[:, sl],
                        channels=128, num_elems=NE_B, num_idxs=M)

                    box_ap = box[:]
                    part_dim = list(box_ap.ap)[0]

                    # center cells (x=5,y=5) -> ctr_g columns
                    ctr_ps = psS.tile([22, 128], F32, tag="small", name="ctr_ps")
                    ctr_in = AP(box_ap.tensor, box_ap.offset + 1320,
                                [part_dim, [1, 22]])
                    nc.tensor.matmul(ctr_ps[:], ctr_in, id_t[:], start=True, stop=True)
                    nc.vector.tensor_copy(ctr_g[:, j * 128:(j + 1) * 128], ctr_ps[:])

                    # 45 im2col chunk transposes, 8 per fp16 psum bank
                    psb = None
                    for k in range(45):
                        p, dy = divmod(k, 5)
                        ox, oy = divmod(p, 3)
                        if k % 4 == 0:
                            psb = psT.tile([128, 512], F32, tag="T", name="psb")
                        # transpose as a REGULAR matmul (chunk.T @ I) so the
                        # PE HAM clock-boost engages (transpose-mode never
                        # warms); M padded to 128 for fast weight load.
                        chunk = AP(box_ap.tensor,
                                   box_ap.offset + 726 * oy + 242 * dy + 66 * ox,
                                   [part_dim, [1, 128]])
                        nc.tensor.matmul(
                            psb[:, (k % 4) * 128:(k % 4 + 1) * 128], chunk,
                            id_t[:], start=True, stop=True)
                        if k % 4 == 3 or k == 44:
                            b = k // 4
                            nk = k % 4 + 1
                            imct = imc[:]
                            dst = AP(imct.tensor, imct.offset + (b * 4) * 512 + j * 128,
                                     [list(imct.ap)[0], [512, nk], [1, 128]])
                            if (j + b) % 2 == 0:
                                nc.vector.tensor_copy(dst, psb[:110, :nk * 128])
                            else:
                                nc.scalar.copy(dst, psb[:110, :nk * 128])

                # --- self encoder (N=512)
                hidA = pa.tile([128, 512], F16, tag="hidA", name="hidA")
                hidB = pa.tile([128, 512], F16, tag="hidB", name="hidB")
                hidC = pa.tile([128, 512], F16, tag="hidC", name="hidC")
                hidD = pa.tile([128, 512], F16, tag="hidD", name="hidD")
                for h, hout in ((0, hidA), (1, hidB)):
                    sf_ps = psS.tile([128, 512], F32, tag="small", name="sf_ps")
                    nc.tensor.matmul(sf_ps[:], sw_t[:, h * 128:(h + 1) * 128],
                                     ctr_g[:], start=True, stop=True)
                    nc.scalar.activation(hout[:], sf_ps[:],
                                         mybir.ActivationFunctionType.Relu,
                                         bias=bself_t[:, h:h + 1])

                # --- conv1 (45 matmuls at N=512) + relu
                c1s = pa.tile([128, 9 * 512], F16, tag="c1s", name="c1s")
                for p in range(9):
                    c1_ps = psC.tile([128, 512], F32, tag="c1", name="c1_ps")
                    for dy in range(5):
                        k = p * 5 + dy
                        nc.tensor.matmul(
                            c1_ps[:], w1_t[:, dy * 128:(dy + 1) * 128],
                            imc[:, k * 512:(k + 1) * 512],
                            start=(dy == 0), stop=(dy == 4))
                    if p % 2 == 0:
                        nc.scalar.activation(
                            c1s[:, p * 512:(p + 1) * 512], c1_ps[:],
                            mybir.ActivationFunctionType.Relu, bias=bc1_t[:, 0:1])
                    else:
                        nc.vector.tensor_scalar(
                            c1s[:, p * 512:(p + 1) * 512], c1_ps[:],
                            bc1_t[:, 0:1], 0.0, op.add, op.max)

                # --- conv2 (9 accumulating matmuls at N=512) + relu
                c2_ps = psS.tile([128, 512], F32, tag="small", name="c2_ps")
                for j2 in range(9):
                    nc.tensor.matmul(c2_ps[:], w2_t[:, j2 * 128:(j2 + 1) * 128],
                                     c1s[:, j2 * 512:(j2 + 1) * 512],
                                     start=(j2 == 0), stop=(j2 == 8))
                c2s = pa.tile([128, 512], F16, tag="c2s", name="c2s")
                nc.vector.tensor_scalar(c2s[:], c2_ps[:], bc2_t[:, 0:1], 0.0,
                                        op.add, op.max)

                # --- fc + relu
                for h, hout in ((0, hidC), (1, hidD)):
                    fc_ps = psS.tile([128, 512], F32, tag="small", name="fc_ps")
                    nc.tensor.matmul(fc_ps[:], fc_t[:, h * 128:(h + 1) * 128],
                                     c2s[:], start=True, stop=True)
                    nc.scalar.activation(hout[:], fc_ps[:],
                                         mybir.ActivationFunctionType.Relu,
                                         bias=bfc_t[:, h:h + 1])

                # --- heads (N=512) -> out[20, rows]
                h_ps = psS.tile([20, 512], F32, tag="small", name="h_ps")
                for kk, hin in enumerate((hidA, hidB, hidC, hidD)):
                    nc.tensor.matmul(h_ps[:], hw_t[:, kk * 20:(kk + 1) * 20],
                                     hin[:], start=(kk == 0), stop=(kk == 3))
                outT = pa.tile([20, 512], F32, tag="outT", name="outT")
                nc.vector.tensor_scalar(outT[:], h_ps[:], bhead_t[:, 0:1],
                                        None, op.add)
                nc.sync.dma_start(out_d[:, g * 512:(g + 1) * 512], outT[:])

    nc.compile()
    return nc


def _consts(inputs):
    f16 = np.float16
    c1w = np.asarray(inputs["conv1_w"], np.float32)      # [128, 22, 5, 5]
    w1c = np.zeros((110, 640), f16)
    for a in range(22):
        for dx in range(5):
            for dy in range(5):
                w1c[dx * 22 + a, dy * 128:(dy + 1) * 128] = (
                    c1w[:, a, dx, dy] / _DIV[a]).astype(f16)
    c2w = np.asarray(inputs["conv2_w"], np.float32)      # [128, 128, 3, 3]
    w2c = np.zeros((128, 1152), f16)
    for kx in range(3):
        for ky in range(3):
            j = kx * 3 + ky
            w2c[:, j * 128:(j + 1) * 128] = c2w[:, :, kx, ky].T.astype(f16)
    fcw = np.asarray(inputs["fc_w"], np.float32)         # [256, 128]
    fcwc = fcw.T.astype(f16)                             # [128, 256]
    sw = np.asarray(inputs["self_w"], np.float32)        # [256, 22]
    selfwc = (sw / _DIV[None, :]).T.astype(f16)          # [22, 256]
    hw = np.concatenate([np.asarray(inputs["actor0_w"]),
                         np.asarray(inputs["actor1_w"]),
                         np.asarray(inputs["value_w"])], axis=0)  # [20, 512]
    headwc = np.zeros((128, 80), f16)
    for kk in range(4):
        headwc[:, kk * 20:(kk + 1) * 20] = hw[:, kk * 128:(kk + 1) * 128].T.astype(f16)
    bhead = np.concatenate([np.asarray(inputs["actor0_b"]),
                            np.asarray(inputs["actor1_b"]),
                            np.asarray(inputs["value_b"])])[:, None].astype(np.float32)
    bfc = np.asarray(inputs["fc_b"], np.float32).reshape(2, 128).T.copy()
    bself = np.asarray(inputs["self_b"], np.float32).reshape(2, 128).T.copy()
    return {
        "w1c": w1c, "w2c": w2c, "fcwc": fcwc, "selfwc": selfwc,
        "headwc": headwc, "identc": np.eye(128, dtype=f16),
        "bc1": np.asarray(inputs["conv1_b"], np.float32)[:, None],
        "bc2": np.asarray(inputs["conv2_b"], np.float32)[:, None],
        "bfc": np.ascontiguousarray(bfc), "bself": np.ascontiguousarray(bself),
        "bhead": bhead,
    }


_NC = None


def _get_nc():
    global _NC
    if _NC is None:
        _NC = _build()
    return _NC


def _run(inputs, trace=False, tmpdir=None):
    nc = _get_nc()
    consts = _consts(inputs)
    obs = np.ascontiguousarray(
        np.asarray(inputs["observations"], np.int32).reshape(B, 3 * M))
    in_maps = []
    for c in range(N_CORES):
        m = dict(consts)
        m["obs"] = obs[c * B_C:(c + 1) * B_C]
        in_maps.append(m)
    res = bass_utils.run_bass_kernel_spmd(
        nc, in_maps, core_ids=list(range(N_CORES)), trace=trace, tmpdir=tmpdir)
    out = np.concatenate([res.results[c]["out"].T for c in range(N_CORES)], axis=0)
    a0 = np.ascontiguousarray(out[:, 0:9], dtype=np.float32)
    a1 = np.ascontiguousarray(out[:, 9:19], dtype=np.float32)
    v = np.ascontiguousarray(out[:, 19:20], dtype=np.float32)
    return (a0, a1, v), res


def kernel(**inputs):
    (a0, a1, v), _ = _run(inputs)
    return a0, a1, v


# revision 11
# speedup vs baseline: 1.6528x; 1.2222x over previous
"""Trainium2 Bass kernel for nn_Policy_31104153158020.

Policy network: token parse -> scatter into [22,11,11] grid -> CNN trunk
(conv 5x5 s3 -> conv 3x3 -> fc) + self encoder on center cell -> 3 heads.

Strategy (pure data parallel over 8 cores, B=16384 -> 2048 rows/core):
- Parse tokens on DVE (bit ops) in groups of 4 row-tiles.
- Scatter per 128-row tile with GPSIMD local_scatter (two overlapping
  halves; HW processes indices in token order => last-write-wins,
  matching XLA scatter semantics exactly).
- conv1 as 45 strided PE transposes (im2col chunks [a,dx] x 5 dy) feeding
  45 fp16 matmuls; conv2/fc/self/heads as plain matmuls; normalization
  (1/max_vec) folded into conv1/self weights on the host.
"""
import os
import sys

sys.path.insert(0, "/opt/trn_rl_repo")

import numpy as np

import concourse.bass as bass
import concourse.bacc as bacc
import concourse.mybir as mybir
import concourse.tile as tile
from concourse import bass_utils

from concourse.ap import AP
from concourse.alu_op_type import AluOpType as op

N_CORES = 8
B = 16384
M = 200
B_C = B // N_CORES          # 2048 rows per core
TILES = B_C // 128          # 16 tiles of 128 rows
GROUP = 4                   # tiles parsed together
N_GROUPS = TILES // GROUP

NUM_LAYERS, OW, OH = 22, 11, 11
NCELL = NUM_LAYERS * OW * OH          # 2662
# scatter split: A covers cells [0,1332) (+dump 1332), B covers [1332,2662)
NE_A = 1334
NE_B = 1334
BOXW = 2880                           # >= 1462 + 11*128; box[:, c] = cell c

F16 = mybir.dt.float16
F32 = mybir.dt.float32
I16 = mybir.dt.int16
I32 = mybir.dt.int32

_MAX_VEC = np.maximum(np.array([9.0, 1.0, 1.0, 10.0, 3.0, 254.0, 1.0, 1.0, 235.0, 8.0,
                                9.0, 250.0, 29.0, 1.0, 1.0, 8.0, 1.0, 1.0, 6.0, 3.0,
                                1.0, 2.0], dtype=np.float32), 1.0)
_DIV = _MAX_VEC + 1e-8


def _build():
    nc = bacc.Bacc("TRN2", target_bir_lowering=False, debug=False,
                   num_devices=N_CORES)

    obs_d = nc.dram_tensor("obs", [B_C, 3 * M], I32, kind="ExternalInput").ap()
    w1_d = nc.dram_tensor("w1c", [110, 640], F16, kind="ExternalInput").ap()
    w2_d = nc.dram_tensor("w2c", [128, 1152], F16, kind="ExternalInput").ap()
    fc_d = nc.dram_tensor("fcwc", [128, 256], F16, kind="ExternalInput").ap()
    sw_d = nc.dram_tensor("selfwc", [22, 256], F16, kind="ExternalInput").ap()
    hw_d = nc.dram_tensor("headwc", [128, 80], F16, kind="ExternalInput").ap()
    id_d = nc.dram_tensor("identc", [128, 128], F16, kind="ExternalInput").ap()
    bc1_d = nc.dram_tensor("bc1", [128, 1], F32, kind="ExternalInput").ap()
    bc2_d = nc.dram_tensor("bc2", [128, 1], F32, kind="ExternalInput").ap()
    bfc_d = nc.dram_tensor("bfc", [128, 2], F32, kind="ExternalInput").ap()
    bself_d = nc.dram_tensor("bself", [128, 2], F32, kind="ExternalInput").ap()
    bhead_d = nc.dram_tensor("bhead", [20, 1], F32, kind="ExternalInput").ap()
    out_d = nc.dram_tensor("out", [20, B_C], F32, kind="ExternalOutput").ap()

    with tile.TileContext(nc) as tc:
        with (
            tc.tile_pool(name="const", bufs=1) as pc,
            tc.tile_pool(name="grp", bufs=2) as pg,
            tc.tile_pool(name="tok", bufs=2) as pt,
            tc.tile_pool(name="boxp", bufs=3) as pb,
            tc.tile_pool(name="imc", bufs=2) as pi,
            tc.tile_pool(name="act", bufs=2) as pa,
            tc.tile_pool(name="psT", bufs=3, space="PSUM") as psT,
            tc.tile_pool(name="psC", bufs=2, space="PSUM") as psC,
            tc.tile_pool(name="psS", bufs=3, space="PSUM") as psS,
        ):
            w1_t = pc.tile([110, 640], F16)
            w2_t = pc.tile([128, 1152], F16)
            fc_t = pc.tile([128, 256], F16)
            sw_t = pc.tile([22, 256], F16)
            hw_t = pc.tile([128, 80], F16)
            id_t = pc.tile([128, 128], F16)
            bc1_t = pc.tile([128, 1], F32)
            bc2_t = pc.tile([128, 1], F32)
            bfc_t = pc.tile([128, 2], F32)
            bself_t = pc.tile([128, 2], F32)
            bhead_t = pc.tile([20, 1], F32)
            nc.sync.dma_start(w1_t[:], w1_d)
            nc.sync.dma_start(w2_t[:], w2_d)
            nc.sync.dma_start(fc_t[:], fc_d)
            nc.sync.dma_start(sw_t[:], sw_d)
            nc.sync.dma_start(hw_t[:], hw_d)
            nc.sync.dma_start(id_t[:], id_d)
            nc.sync.dma_start(bc1_t[:], bc1_d)
            nc.sync.dma_start(bc2_t[:], bc2_d)
            nc.sync.dma_start(bfc_t[:], bfc_d)
            nc.sync.dma_start(bself_t[:], bself_d)
            nc.sync.dma_start(bhead_t[:], bhead_d)

            for g in range(N_GROUPS):
                FD = GROUP * M
                obs_g = pg.tile([128, GROUP * 3 * M], I32, tag="obs")
                for j in range(GROUP):
                    t0 = g * GROUP + j
                    nc.sync.dma_start(
                        obs_g[:, j * 3 * M:(j + 1) * 3 * M],
                        obs_d[t0 * 128:(t0 + 1) * 128, :],
                    )

                # token fields, strided int32 -> int16 casts
                c16 = pt.tile([128, FD], I16, tag="c16")
                a16 = pt.tile([128, FD], I16, tag="a16")
                vals = pt.tile([128, FD], F16, tag="vals")
                obs_v = obs_g[:]
                nc.vector.tensor_copy(c16[:], obs_v[:, 0::3])
                nc.vector.tensor_copy(a16[:], obs_v[:, 1::3])
                nc.vector.tensor_copy(vals[:], obs_v[:, 2::3])

                y16 = pt.tile([128, FD], I16, tag="y16")
                x16 = pt.tile([128, FD], I16, tag="x16")
                t1 = pt.tile([128, FD], I16, tag="t1")
                cell = pt.tile([128, FD], I16, tag="cell")
                m0 = pt.tile([128, FD], I16, tag="m0")
                m1 = pt.tile([128, FD], I16, tag="m1")
                idxA = pt.tile([128, FD], I16, tag="idxA")
                idxB = pt.tile([128, FD], I16, tag="idxB")

                nc.vector.tensor_scalar(y16[:], c16[:], 15, None, op.bitwise_and)
                nc.vector.tensor_scalar(x16[:], c16[:], 4, None, op.logical_shift_right)
                # cell = 242*y + 22*x + a  (im2col permutation: chunk
                # (p,dy) = one CONTIGUOUS 110-run over (dx, a) -> FWL)
                nc.vector.tensor_scalar(t1[:], y16[:], 242, None, op.mult)
                nc.vector.tensor_scalar(cell[:], x16[:], 22, None, op.mult)
                nc.vector.tensor_tensor(cell[:], cell[:], t1[:], op.add)
                nc.vector.tensor_tensor(cell[:], cell[:], a16[:], op.add)
                # validity mask
                nc.vector.tensor_scalar(m0[:], x16[:], 11, None, op.is_lt)
                nc.vector.tensor_scalar(m1[:], y16[:], 11, None, op.is_lt)
                nc.vector.tensor_tensor(m0[:], m0[:], m1[:], op.mult)
                nc.vector.tensor_scalar(m1[:], a16[:], 22, None, op.is_lt)
                nc.vector.tensor_tensor(m0[:], m0[:], m1[:], op.mult)
                # invalid -> cell + 16384 (wraps to negative for large cells)
                nc.vector.tensor_scalar(m0[:], m0[:], -16384, 16384, op.mult, op.add)
                nc.vector.tensor_tensor(cell[:], cell[:], m0[:], op.add)
                # split indices for the two scatter halves
                nc.vector.tensor_scalar(idxA[:], cell[:], 1332, None, op.min)
                nc.vector.tensor_scalar(idxB[:], cell[:], 2664, 1332, op.min, op.subtract)

                # --- scatter + transposes per row-tile; conv at N=512 ---
                imc = pi.tile([110, 45 * 512], F16, tag="imc", name="imc")
                ctr_g = pa.tile([22, 512], F16, tag="ctr_g", name="ctr_g")
                for j in range(GROUP):
                    t0 = g * GROUP + j
                    sl = slice(j * M, (j + 1) * M)
                    box = pb.tile([128, BOXW], F16, tag="box", name="box")
                    nc.gpsimd.local_scatter(
                        box[:, 0:NE_A], vals[:, sl], idxA[:, sl],
                        channels=128, num_elems=NE_A, num_idxs=M)
                    nc.gpsimd.local_scatter(
                        box[:, 1332:1332 + NE_B], vals[:, sl], idxB[:, sl],
                        channels=128, num_elems=NE_B, num_idxs=M)

                    box_ap = box[:]
                    part_dim = list(box_ap.ap)[0]

                    # center cells (x=5,y=5) -> ctr_g columns
                    ctr_ps = psS.tile([22, 128], F32, tag="small", name="ctr_ps")
                    ctr_in = AP(box_ap.tensor, box_ap.offset + 1320,
                                [part_dim, [1, 22]])
                    nc.tensor.matmul(ctr_ps[:], ctr_in, id_t[:], start=True, stop=True)
                    nc.vector.tensor_copy(ctr_g[:, j * 128:(j + 1) * 128], ctr_ps[:])

                    # 45 im2col chunk transposes, 8 per fp16 psum bank
                    psb = None
                    for k in range(45):
                        p, dy = divmod(k, 5)
                        ox, oy = divmod(p, 3)
                        if k % 4 == 0:
                            psb = psT.tile([128, 512], F32, tag="T", name="psb")
                        # transpose as a REGULAR matmul (chunk.T @ I) so the
                        # PE HAM clock-boost engages (transpose-mode never
                        # warms); M padded to 128 for fast weight load.
                        chunk = AP(box_ap.tensor,
                                   box_ap.offset + 726 * oy + 242 * dy + 66 * ox,
                                   [part_dim, [1, 128]])
                        nc.tensor.matmul(
                            psb[:, (k % 4) * 128:(k % 4 + 1) * 128], chunk,
                            id_t[:], start=True, stop=True)
                        if k % 4 == 3 or k == 44:
                            b = k // 4
                            nk = k % 4 + 1
                            imct = imc[:]
                            dst = AP(imct.tensor, imct.offset + (b * 4) * 512 + j * 128,
                                     [list(imct.ap)[0], [512, nk], [1, 128]])
                            if (j + b) % 3 == 0:
                                nc.vector.tensor_copy(dst, psb[:110, :nk * 128])
                            else:
                                nc.scalar.copy(dst, psb[:110, :nk * 128])

                # --- self encoder (N=512)
                hidA = pa.tile([128, 512], F16, tag="hidA", name="hidA")
                hidB = pa.tile([128, 512], F16, tag="hidB", name="hidB")
                hidC = pa.tile([128, 512], F16, tag="hidC", name="hidC")
                hidD = pa.tile([128, 512], F16, tag="hidD", name="hidD")
                for h, hout in ((0, hidA), (1, hidB)):
                    sf_ps = psS.tile([128, 512], F32, tag="small", name="sf_ps")
                    nc.tensor.matmul(sf_ps[:], sw_t[:, h * 128:(h + 1) * 128],
                                     ctr_g[:], start=True, stop=True)
                    nc.scalar.activation(hout[:], sf_ps[:],
                                         mybir.ActivationFunctionType.Relu,
                                         bias=bself_t[:, h:h + 1])

                # --- conv1 (45 matmuls at N=512) + relu
                c1s = pa.tile([128, 9 * 512], F16, tag="c1s", name="c1s")
                for p in range(9):
                    c1_ps = psC.tile([128, 512], F32, tag="c1", name="c1_ps")
                    for dy in range(5):
                        k = p * 5 + dy
                        nc.tensor.matmul(
                            c1_ps[:], w1_t[:, dy * 128:(dy + 1) * 128],
                            imc[:, k * 512:(k + 1) * 512],
                            start=(dy == 0), stop=(dy == 4))
                    if p % 2 == 0:
                        nc.scalar.activation(
                            c1s[:, p * 512:(p + 1) * 512], c1_ps[:],
                            mybir.ActivationFunctionType.Relu, bias=bc1_t[:, 0:1])
                    else:
                        nc.vector.tensor_scalar(
                            c1s[:, p * 512:(p + 1) * 512], c1_ps[:],
                            bc1_t[:, 0:1], 0.0, op.add, op.max)

                # --- conv2 (9 accumulating matmuls at N=512) + relu
                c2_ps = psS.tile([128, 512], F32, tag="small", name="c2_ps")
                for j2 in range(9):
                    nc.tensor.matmul(c2_ps[:], w2_t[:, j2 * 128:(j2 + 1) * 128],
                                     c1s[:, j2 * 512:(j2 + 1) * 512],
                                     start=(j2 == 0), stop=(j2 == 8))
                c2s = pa.tile([128, 512], F16, tag="c2s", name="c2s")
                nc.vector.tensor_scalar(c2s[:], c2_ps[:], bc2_t[:, 0:1], 0.0,
                                        op.add, op.max)

                # --- fc + relu
                for h, hout in ((0, hidC), (1, hidD)):
                    fc_ps = psS.tile([128, 512], F32, tag="small", name="fc_ps")
                    nc.tensor.matmul(fc_ps[:], fc_t[:, h * 128:(h + 1) * 128],
                                     c2s[:], start=True, stop=True)
                    nc.scalar.activation(hout[:], fc_ps[:],
                                         mybir.ActivationFunctionType.Relu,
                                         bias=bfc_t[:, h:h + 1])

                # --- heads (N=512) -> out[20, rows]
                h_ps = psS.tile([20, 512], F32, tag="small", name="h_ps")
                for kk, hin in enumerate((hidA, hidB, hidC, hidD)):
                    nc.tensor.matmul(h_ps[:], hw_t[:, kk * 20:(kk + 1) * 20],
                                     hin[:], start=(kk == 0), stop=(kk == 3))
                outT = pa.tile([20, 512], F32, tag="outT", name="outT")
                nc.vector.tensor_scalar(outT[:], h_ps[:], bhead_t[:, 0:1],
                                        None, op.add)
                nc.sync.dma_start(out_d[:, g * 512:(g + 1) * 512], outT[:])

    nc.compile()
    return nc


def _consts(inputs):
    f16 = np.float16
    c1w = np.asarray(inputs["conv1_w"], np.float32)      # [128, 22, 5, 5]
    w1c = np.zeros((110, 640), f16)
    for a in range(22):
        for dx in range(5):
            for dy in range(5):
                w1c[dx * 22 + a, dy * 128:(dy + 1) * 128] = (
                    c1w[:, a, dx, dy] / _DIV[a]).astype(f16)
    c2w = np.asarray(inputs["conv2_w"], np.float32)      # [128, 128, 3, 3]
    w2c = np.zeros((128, 1152), f16)
    for kx in range(3):
        for ky in range(3):
            j = kx * 3 + ky
            w2c[:, j * 128:(j + 1) * 128] = c2w[:, :, kx, ky].T.astype(f16)
    fcw = np.asarray(inputs["fc_w"], np.float32)         # [256, 128]
    fcwc = fcw.T.astype(f16)                             # [128, 256]
    sw = np.asarray(inputs["self_w"], np.float32)        # [256, 22]
    selfwc = (sw / _DIV[None, :]).T.astype(f16)          # [22, 256]
    hw = np.concatenate([np.asarray(inputs["actor0_w"]),
                         np.asarray(inputs["actor1_w"]),
                         np.asarray(inputs["value_w"])], axis=0)  # [20, 512]
    headwc = np.zeros((128, 80), f16)
    for kk in range(4):
        headwc[:, kk * 20:(kk + 1) * 20] = hw[:, kk * 128:(kk + 1) * 128].T.astype(f16)
    bhead = np.concatenate([np.asarray(inputs["actor0_b"]),
                            np.asarray(inputs["actor1_b"]),
                            np.asarray(inputs["value_b"])])[:, None].astype(np.float32)
    bfc = np.asarray(inputs["fc_b"], np.float32).reshape(2, 128).T.copy()
    bself = np.asarray(inputs["self_b"], np.float32).reshape(2, 128).T.copy()
    return {
        "w1c": w1c, "w2c": w2c, "fcwc": fcwc, "selfwc": selfwc,
        "headwc": headwc, "identc": np.eye(128, dtype=f16),
        "bc1": np.asarray(inputs["conv1_b"], np.float32)[:, None],
        "bc2": np.asarray(inputs["conv2_b"], np.float32)[:, None],
        "bfc": np.ascontiguousarray(bfc), "bself": np.ascontiguousarray(bself),
        "bhead": bhead,
    }


_NC = None


def _get_nc():
    global _NC
    if _NC is None:
        _NC = _build()
    return _NC


def _run(inputs, trace=False, tmpdir=None):
    nc = _get_nc()
    consts = _consts(inputs)
    obs = np.ascontiguousarray(
        np.asarray(inputs["observations"], np.int32).reshape(B, 3 * M))
    in_maps = []
    for c in range(N_CORES):
        m = dict(consts)
        m["obs"] = obs[c * B_C:(c + 1) * B_C]
        in_maps.append(m)
    res = bass_utils.run_bass_kernel_spmd(
        nc, in_maps, core_ids=list(range(N_CORES)), trace=trace, tmpdir=tmpdir)
    out = np.concatenate([res.results[c]["out"].T for c in range(N_CORES)], axis=0)
    a0 = np.ascontiguousarray(out[:, 0:9], dtype=np.float32)
    a1 = np.ascontiguousarray(out[:, 9:19], dtype=np.float32)
    v = np.ascontiguousarray(out[:, 19:20], dtype=np.float32)
    return (a0, a1, v), res


def kernel(**inputs):
    (a0, a1, v), _ = _run(inputs)
    return a0, a1, v
